# revision 6
# baseline (speedup 1.0000x reference)
"""Trainium2 Bass kernel for nn_Attention_50654844289068.

Strategy (8 NeuronCores, data-parallel over batch B=8 -> 1 batch element per core):

  reference math per batch b:
    q = query @ Wq.T + bq            (S, 64)
    k = key   @ Wk.T + bk            (S, 64)
    v = value @ Wv.T + bv            (S, 64)
    s = (q @ k.T) * scale            (S, S)
    s = where(s == 0, eps, s); s = where(mask == 0, eps, s)
    w = softmax(s, axis=-1)          (S, S)   <- output 2
    att = w @ v                      (S, 64)  <- output 1

  Device-side layout choices (per core):
    - All big tensors are handled in TRANSPOSED score layout  sT[sk, sq]
      so that softmax's reduction axis (sk) lands on the partition axis,
      where the TensorEngine can reduce it for free via an appended
      ones-column in the attention@V matmul, and the e^T tiles are directly
      usable as the stationary operand of that matmul (no on-chip 2048x2048
      transpose needed).
    - The host pre-transposes query/key/value ( -> [512, S]) and the mask
      ( -> maskT[sk, sq]) while sharding, and post-transposes the weight
      output (device writes w^T).  Host-side work is only layout/dtype prep.
    - masked_fill(s==0, eps) + masked_fill(mask==0, eps):  eps = 1e-6, and
      exp(1e-6) == 1 + 1e-6.  We instead compute e = exp(scale*s*mask) so
      masked lanes give exp(0) = 1 — a 1e-6 relative difference, far below
      tolerance.  softmax has no max-subtraction: scores*scale are O(+-2),
      exp is perfectly stable there (matches jax softmax mathematically).
    - bf16 compute on PE (fp32 matmul is 4x slower), fp32 PSUM accumulate.

  Per-core phases:
    P0: project q^T,k^T [64,S] (bf16) and v [S,64] (+ ones col) from
        host-transposed inputs.
    P1: for each of 16 sk-blocks: scoresT = k_blk^T . q  (PE) ->
        s' = (scores*scale)*mask (DVE, reads PSUM) -> eT = exp(s') (ACT)
        -> accumulate attT[65, S] += [v|1]^T . eT (PE).
    P2: rowsums = attT row 64 -> 1/r (DVE) -> broadcast over partitions via
        a tiny DRAM bounce -> w^T = eT * rinv (DVE) -> DMA out;
        att = (attT rows 0:64 * rinv) transposed back on PE -> DMA out.
"""

import os
import sys
from contextlib import ExitStack

sys.path.insert(0, "/opt/trn_rl_repo")

import numpy as np
import ml_dtypes

import concourse.bacc as bacc
import concourse.bass as bass
import concourse.tile as tile
from concourse import masks, mybir
from concourse.bass_utils import run_bass_kernel_spmd

B, S, DM, DK = 8, 2048, 512, 64
NCORES = 8
P = 128
NKB = S // P          # 16 sk blocks
NCH = S // 512        # 4 sq chunks of 512
SCALE = float(DK) ** -0.5

F32 = mybir.dt.float32
BF16 = mybir.dt.bfloat16
U8 = mybir.dt.uint8
NPBF16 = ml_dtypes.bfloat16

AF = mybir.ActivationFunctionType
OP = mybir.AluOpType

LAST_RESULTS = None


def build_graph():
    nc = bacc.Bacc(
        "TRN2",
        target_bir_lowering=False,
        debug=False,
        num_devices=NCORES,
    )

    qT = nc.declare_dram_parameter("qT", [DM, S], BF16, isOutput=False)
    kT = nc.declare_dram_parameter("kT", [DM, S], BF16, isOutput=False)
    vT = nc.declare_dram_parameter("vT", [DM, S], BF16, isOutput=False)
    maskT = nc.declare_dram_parameter("maskT", [S, S], U8, isOutput=False)
    wTs = {
        t: nc.declare_dram_parameter(f"w{t}T", [DM, DK], BF16, isOutput=False)
        for t in "qkv"
    }
    biases = {
        t: nc.declare_dram_parameter(f"b{t}", [DK, 1], F32, isOutput=False)
        for t in "qkv"
    }
    w_t = nc.declare_dram_parameter("w_t", [S, S], BF16, isOutput=True)
    att = nc.declare_dram_parameter("att", [S, DK], F32, isOutput=True)

    r_scr = nc.dram_tensor("r_scr", [S], F32)
    rinv_scr = nc.dram_tensor("rinv_scr", [S], BF16)

    xTs = {"q": qT, "k": kT, "v": vT}

    with tile.TileContext(nc) as tc, ExitStack() as ctx:
        persist = ctx.enter_context(tc.tile_pool(name="persist", bufs=1))
        et_pool = ctx.enter_context(tc.tile_pool(name="et", bufs=NKB))

        ident_bf16 = persist.tile([P, P], BF16, tag="ident_bf16")
        masks.make_identity(nc, ident_bf16[:, :])
        ident_f32 = persist.tile([P, P], F32, tag="ident_f32")
        masks.make_identity(nc, ident_f32[:, :])

        qT_sb = persist.tile([DK, S], BF16, tag="qT_sb")
        kT_sb = persist.tile([DK, S], BF16, tag="kT_sb")
        v1_tiles = [
            persist.tile([P, DK + 1], BF16, tag=f"v1_{i}", name=f"v1_{i}")
            for i in range(NKB)
        ]

        # ---------------- P0: projections ----------------
        with tc.tile_pool(name="pro_in", bufs=4) as pro_in, \
             tc.tile_pool(name="pro_w", bufs=1) as pro_w, \
             tc.tile_pool(name="pro_tmp", bufs=1) as pro_tmp, \
             tc.tile_pool(name="pro_ps", bufs=2, space="PSUM") as pro_ps, \
             tc.tile_pool(name="tr_ps", bufs=2, space="PSUM") as tr_ps:

            bias_sb = {}
            for t in "qkv":
                bt = pro_w.tile([DK, 1], F32, tag=f"bias_{t}")
                nc.sync.dma_start(bt[:, :], biases[t][:, :])
                bias_sb[t] = bt

            vT_sb = pro_tmp.tile([DK, S], BF16, tag="vT_sb")
            dst = {"q": qT_sb, "k": kT_sb, "v": vT_sb}

            for t in "qkv":
                w_tiles = []
                for m in range(4):
                    wt = pro_w.tile([P, DK], BF16, tag=f"w_{t}_{m}")
                    nc.sync.dma_start(wt[:, :], wTs[t][m * P:(m + 1) * P, :])
                    w_tiles.append(wt)
                x_tiles = []
                for m in range(4):
                    xt = pro_in.tile([P, S], BF16, tag="xin")
                    nc.sync.dma_start(xt[:, :], xTs[t][m * P:(m + 1) * P, :])
                    x_tiles.append(xt)
                for c in range(NCH):
                    ps = pro_ps.tile([DK, 512], F32, tag="proj_ps")
                    for m in range(4):
                        nc.tensor.matmul(
                            ps[:, :],
                            w_tiles[m][:, :],
                            x_tiles[m][:, c * 512:(c + 1) * 512],
                            start=(m == 0),
                            stop=(m == 3),
                        )
                    # PSUM -> SBUF with per-partition bias add, cast to bf16
                    nc.scalar.activation(
                        dst[t][:, c * 512:(c + 1) * 512],
                        ps[:, :],
                        AF.Identity,
                        bias=bias_sb[t][:, :],
                        scale=1.0,
                    )

            # v natural [sk, 64] tiles with an appended ones column
            for i in range(NKB):
                pst = tr_ps.tile([P, DK], BF16, tag="tr_ps")
                nc.tensor.transpose(
                    pst[:, :], vT_sb[:, i * P:(i + 1) * P], ident_bf16[:DK, :DK]
                )
                nc.scalar.copy(v1_tiles[i][:, 0:DK], pst[:, :])
                nc.vector.memset(v1_tiles[i][:, DK:DK + 1], 1.0)

        # ---------------- P1: main loop over sk blocks ----------------
        with tc.tile_pool(name="att_ps", bufs=1, space="PSUM") as att_ps:
            att_acc = [
                att_ps.tile([DK + 1, 512], F32, tag=f"att_acc{c}", name=f"att_acc{c}")
                for c in range(NCH)
            ]
            et_tiles = []
            with tc.tile_pool(name="mask_p", bufs=3) as mask_p, \
                 tc.tile_pool(name="sp_p", bufs=3) as sp_p, \
                 tc.tile_pool(name="sc_ps", bufs=2, space="PSUM") as sc_ps:

                for kb in range(NKB):
                    mtile = mask_p.tile([P, S], U8, tag="mask")
                    nc.sync.dma_start(mtile[:, :], maskT[kb * P:(kb + 1) * P, :])

                    sp = sp_p.tile([P, S], BF16, tag="sp")
                    for h in range(2):
                        ps = sc_ps.tile([P, 1024], F32, tag="sc_ps")
                        for cc in range(2):
                            c = h * 2 + cc
                            nc.tensor.matmul(
                                ps[:, cc * 512:(cc + 1) * 512],
                                kT_sb[:, kb * P:(kb + 1) * P],
                                qT_sb[:, c * 512:(c + 1) * 512],
                            )
                        # s' = (scores * scale) * mask   (PSUM + SBUF -> SBUF bf16)
                        nc.vector.scalar_tensor_tensor(
                            sp[:, h * 1024:(h + 1) * 1024],
                            ps[:, :],
                            SCALE,
                            mtile[:, h * 1024:(h + 1) * 1024],
                            op0=OP.mult,
                            op1=OP.mult,
                        )

                    et = et_pool.tile([P, S], BF16, tag="et")
                    nc.scalar.activation(et[:, :], sp[:, :], AF.Exp, bias=0.0, scale=1.0)
                    et_tiles.append(et)

                    for c in range(NCH):
                        nc.tensor.matmul(
                            att_acc[c][:, :],
                            v1_tiles[kb][:, :],
                            et[:, c * 512:(c + 1) * 512],
                            start=(kb == 0),
                            stop=(kb == NKB - 1),
                        )

            # ---------------- P2: normalize + outputs ----------------
            att_sb = persist.tile([DK + 1, S], F32, tag="att_sb")
            for c in range(NCH):
                nc.scalar.copy(att_sb[:, c * 512:(c + 1) * 512], att_acc[c][:, :])

        with tc.tile_pool(name="ph2", bufs=1) as ph2, \
             tc.tile_pool(name="w_p", bufs=3) as w_p, \
             tc.tile_pool(name="ao_p", bufs=3) as ao_p, \
             tc.tile_pool(name="tr2_ps", bufs=2, space="PSUM") as tr2_ps:

            # rowsums r live in att_sb row 64.  Reshape through DRAM to get
            # them onto 128 partitions, reciprocal, then broadcast-load.
            nc.sync.dma_start(r_scr[:], att_sb[DK:DK + 1, :])
            rr_t = ph2.tile([P, S // P], F32, tag="rr_t")
            nc.sync.dma_start(rr_t[:, :], r_scr[:].rearrange("(p f) -> p f", p=P))
            rr_inv = ph2.tile([P, S // P], BF16, tag="rr_inv")
            with nc.allow_low_precision(reason="bf16 1/rowsum is within tolerance"):
                nc.vector.reciprocal(rr_inv[:, :], rr_t[:, :])
            nc.sync.dma_start(rinv_scr[:].rearrange("(p f) -> p f", p=P), rr_inv[:, :])

            rinv_bc = ph2.tile([P, S], BF16, tag="rinv_bc")
            nc.sync.dma_start(
                rinv_bc[:, :],
                rinv_scr[:].rearrange("(a s) -> a s", a=1).to_broadcast((P, S)),
            )

            # attention weights out: w^T = eT * rinv  (per sk block)
            for kb in range(NKB):
                wsb = w_p.tile([P, S], BF16, tag="wsb")
                nc.vector.tensor_tensor(
                    wsb[:, :], et_tiles[kb][:, :], rinv_bc[:, :], op=OP.mult
                )
                nc.sync.dma_start(w_t[kb * P:(kb + 1) * P, :], wsb[:, :])

            # attention out: normalize attT then transpose back to [sq, 64]
            attn_sb = ph2.tile([DK, S], F32, tag="attn_sb")
            nc.vector.tensor_tensor(
                attn_sb[:, :], att_sb[0:DK, :], rinv_bc[0:DK, :], op=OP.mult
            )
            for i in range(NKB):
                pst = tr2_ps.tile([P, DK], F32, tag="tr2_ps")
                nc.tensor.transpose(
                    pst[:, :], attn_sb[:, i * P:(i + 1) * P], ident_f32[:DK, :DK]
                )
                ao = ao_p.tile([P, DK], F32, tag="ao")
                nc.scalar.copy(ao[:, :], pst[:, :])
                nc.sync.dma_start(att[i * P:(i + 1) * P, :], ao[:, :])

    nc.finalize()
    return nc


_CACHE = {}


def _get_graph():
    if "nc" not in _CACHE:
        _CACHE["nc"] = build_graph()
    return _CACHE["nc"]


def make_in_maps(query, key, value, attention_mask, Wq, bq, Wk, bk, Wv, bv):
    query = np.asarray(query)
    key = np.asarray(key)
    value = np.asarray(value)
    attention_mask = np.asarray(attention_mask)
    shared = {
        "wqT": np.ascontiguousarray(np.asarray(Wq, np.float32).T).astype(NPBF16),
        "wkT": np.ascontiguousarray(np.asarray(Wk, np.float32).T).astype(NPBF16),
        "wvT": np.ascontiguousarray(np.asarray(Wv, np.float32).T).astype(NPBF16),
        "bq": np.asarray(bq, np.float32).reshape(DK, 1),
        "bk": np.asarray(bk, np.float32).reshape(DK, 1),
        "bv": np.asarray(bv, np.float32).reshape(DK, 1),
    }
    in_maps = []
    for b in range(B):
        in_maps.append(
            {
                "qT": np.ascontiguousarray(query[b].T).astype(NPBF16),
                "kT": np.ascontiguousarray(key[b].T).astype(NPBF16),
                "vT": np.ascontiguousarray(value[b].T).astype(NPBF16),
                "maskT": np.ascontiguousarray(attention_mask[b].T).astype(np.uint8),
                **shared,
            }
        )
    return in_maps


def kernel(query, key, value, attention_mask, Wq, bq, Wk, bk, Wv, bv):
    global LAST_RESULTS
    nc = _get_graph()
    in_maps = make_in_maps(
        query, key, value, attention_mask, Wq, bq, Wk, bk, Wv, bv
    )
    res = run_bass_kernel_spmd(nc, in_maps, core_ids=list(range(NCORES)))
    LAST_RESULTS = res
    att = np.stack(
        [np.asarray(res.results[c]["att"], np.float32) for c in range(NCORES)]
    )
    w = np.stack(
        [
            np.asarray(res.results[c]["w_t"]).astype(np.float32).T
            for c in range(NCORES)
        ]
    )
    return att, np.ascontiguousarray(w)


# revision 10
# speedup vs baseline: 1.1775x; 1.1775x over previous
"""Trainium2 Bass kernel for nn_Attention_50654844289068.

Strategy (8 NeuronCores, data-parallel over batch B=8 -> 1 batch element per core):

  reference math per batch b:
    q = query @ Wq.T + bq            (S, 64)
    k = key   @ Wk.T + bk            (S, 64)
    v = value @ Wv.T + bv            (S, 64)
    s = (q @ k.T) * scale            (S, S)
    s = where(s == 0, eps, s); s = where(mask == 0, eps, s)
    w = softmax(s, axis=-1)          (S, S)   <- output 2
    att = w @ v                      (S, 64)  <- output 1

  Device-side layout choices (per core):
    - All big tensors are handled in TRANSPOSED score layout  sT[sk, sq]
      so that softmax's reduction axis (sk) lands on the partition axis,
      where the TensorEngine can reduce it for free via an appended
      ones-column in the attention@V matmul, and the e^T tiles are directly
      usable as the stationary operand of that matmul (no on-chip 2048x2048
      transpose needed).
    - The host pre-transposes query/key/value ( -> [512, S]) and the mask
      ( -> maskT[sk, sq]) while sharding, and post-transposes the weight
      output (device writes w^T).  Host-side work is only layout/dtype prep.
    - masked_fill(s==0, eps) + masked_fill(mask==0, eps):  eps = 1e-6, and
      exp(1e-6) == 1 + 1e-6.  We instead compute e = exp(scale*s*mask) so
      masked lanes give exp(0) = 1 — a 1e-6 relative difference, far below
      tolerance.  softmax has no max-subtraction: scores*scale are O(+-2),
      exp is perfectly stable there (matches jax softmax mathematically).
    - bf16 compute on PE (fp32 matmul is 4x slower), fp32 PSUM accumulate.

  Per-core phases:
    P0: project q^T,k^T [64,S] (bf16) and v [S,64] (+ ones col) from
        host-transposed inputs.
    P1: for each of 16 sk-blocks: scoresT = k_blk^T . q  (PE) ->
        s' = (scores*scale)*mask (DVE, reads PSUM) -> eT = exp(s') (ACT)
        -> accumulate attT[65, S] += [v|1]^T . eT (PE).
    P2: rowsums = attT row 64 -> 1/r (DVE) -> broadcast over partitions via
        a tiny DRAM bounce -> w^T = eT * rinv (DVE) -> DMA out;
        att = (attT rows 0:64 * rinv) transposed back on PE -> DMA out.
"""

import os
import sys
from contextlib import ExitStack

sys.path.insert(0, "/opt/trn_rl_repo")

import numpy as np
import ml_dtypes

import concourse.bacc as bacc
import concourse.bass as bass
import concourse.tile as tile
from concourse import masks, mybir
from concourse.bass_utils import run_bass_kernel_spmd

B, S, DM, DK = 8, 2048, 512, 64
NCORES = 8
P = 128
NKB = S // P          # 16 sk blocks
NCH = S // 512        # 4 sq chunks of 512
SCALE = float(DK) ** -0.5

F32 = mybir.dt.float32
BF16 = mybir.dt.bfloat16
U8 = mybir.dt.uint8
NPBF16 = ml_dtypes.bfloat16

AF = mybir.ActivationFunctionType
OP = mybir.AluOpType

LAST_RESULTS = None


def build_graph():
    nc = bacc.Bacc(
        "TRN2",
        target_bir_lowering=False,
        debug=False,
        num_devices=NCORES,
    )

    qT = nc.declare_dram_parameter("qT", [DM, S], BF16, isOutput=False)
    kT = nc.declare_dram_parameter("kT", [DM, S], BF16, isOutput=False)
    vT = nc.declare_dram_parameter("vT", [DM, S], BF16, isOutput=False)
    maskT = nc.declare_dram_parameter("maskT", [S, S], U8, isOutput=False)
    wTs = {
        t: nc.declare_dram_parameter(f"w{t}T", [DM, DK], BF16, isOutput=False)
        for t in "qkv"
    }
    biases = {
        t: nc.declare_dram_parameter(f"b{t}", [DK, 1], F32, isOutput=False)
        for t in "qkv"
    }
    w_t = nc.declare_dram_parameter("w_t", [S, S], BF16, isOutput=True)
    att = nc.declare_dram_parameter("att", [S, DK], F32, isOutput=True)

    r_scr = nc.dram_tensor("r_scr", [S], F32)
    rinv_scr = nc.dram_tensor("rinv_scr", [S], BF16)

    xTs = {"q": qT, "k": kT, "v": vT}

    with tile.TileContext(nc) as tc, ExitStack() as ctx:
        persist = ctx.enter_context(tc.tile_pool(name="persist", bufs=1))
        et_pool = ctx.enter_context(tc.tile_pool(name="et", bufs=2 * NKB))

        ident_bf16 = persist.tile([P, P], BF16, tag="ident_bf16")
        masks.make_identity(nc, ident_bf16[:, :])
        ident_f32 = persist.tile([P, P], F32, tag="ident_f32")
        masks.make_identity(nc, ident_f32[:, :])

        qT_sb = persist.tile([DK, S], BF16, tag="qT_sb")
        kT_sb = persist.tile([DK, S], BF16, tag="kT_sb")
        v1_tiles = [
            persist.tile([P, DK + 1], BF16, tag=f"v1_{i}", name=f"v1_{i}")
            for i in range(NKB)
        ]

        # ---------------- P0: projections ----------------
        with tc.tile_pool(name="pro_in", bufs=4) as pro_in, \
             tc.tile_pool(name="pro_w", bufs=1) as pro_w, \
             tc.tile_pool(name="pro_tmp", bufs=1) as pro_tmp, \
             tc.tile_pool(name="pro_ps", bufs=4, space="PSUM") as pro_ps, \
             tc.tile_pool(name="tr_ps", bufs=2, space="PSUM") as tr_ps:

            bias_sb = {}
            for t in "qkv":
                bt = pro_w.tile([DK, 1], F32, tag=f"bias_{t}")
                nc.sync.dma_start(bt[:, :], biases[t][:, :])
                bias_sb[t] = bt

            vT_sb = pro_tmp.tile([DK, S], BF16, tag="vT_sb")
            dst = {"q": qT_sb, "k": kT_sb, "v": vT_sb}

            for t in "qkv":
                w_tiles = []
                for m in range(4):
                    wt = pro_w.tile([P, DK], BF16, tag=f"w_{t}_{m}")
                    nc.sync.dma_start(wt[:, :], wTs[t][m * P:(m + 1) * P, :])
                    w_tiles.append(wt)
                # 4 chunk-PSUMs stay live; m-tiles stream through 2 slots so
                # matmuls start as soon as each m-slice of the input lands.
                pss = [pro_ps.tile([DK, 512], F32, tag="proj_ps", name=f"ps_{t}_{c}")
                       for c in range(NCH)]
                for m in range(4):
                    xt = pro_in.tile([P, S], BF16, tag="xin")
                    nc.sync.dma_start(xt[:, :], xTs[t][m * P:(m + 1) * P, :])
                    for c in range(NCH):
                        nc.tensor.matmul(
                            pss[c][:, :],
                            w_tiles[m][:, :],
                            xt[:, c * 512:(c + 1) * 512],
                            start=(m == 0),
                            stop=(m == 3),
                        )
                for c in range(NCH):
                    nc.scalar.activation(
                        dst[t][:, c * 512:(c + 1) * 512],
                        pss[c][:, :],
                        AF.Identity,
                        bias=bias_sb[t][:, :],
                        scale=1.0,
                    )

            # v natural [sk, 64] tiles with an appended ones column
            for i in range(NKB):
                pst = tr_ps.tile([P, DK], BF16, tag="tr_ps")
                nc.tensor.transpose(
                    pst[:, :], vT_sb[:, i * P:(i + 1) * P], ident_bf16[:DK, :DK]
                )
                nc.scalar.copy(v1_tiles[i][:, 0:DK], pst[:, :])
                nc.vector.memset(v1_tiles[i][:, DK:DK + 1], 1.0)

            # C[d] = sum_sk v[sk, d]  (the +1 correction for g = e - 1;
            # C[64] = S).  Free-axis reduction over vT_sb on DVE.
            c_sb = persist.tile([DK + 1, 1], F32, tag="c_sb")
            nc.vector.reduce_sum(c_sb[0:DK, :], vT_sb[:, :],
                                 axis=mybir.AxisListType.X)
            nc.vector.memset(c_sb[DK:DK + 1, :], float(S))

        # ---------------- main: two sq halves, pipelined ----------------
        H = S // 2          # 1024 columns (sq) per half
        HCH = H // 512      # 2 chunks of 512
        with tc.tile_pool(name="att_ps", bufs=1, space="PSUM") as att_ps, \
             tc.tile_pool(name="mask_p", bufs=4) as mask_p, \
             tc.tile_pool(name="er_p", bufs=3) as er_p, \
             tc.tile_pool(name="sc_ps", bufs=2, space="PSUM") as sc_ps, \
             tc.tile_pool(name="ph2", bufs=2) as ph2, \
             tc.tile_pool(name="w_p", bufs=4) as w_p, \
             tc.tile_pool(name="ao_p", bufs=3) as ao_p, \
             tc.tile_pool(name="tr2_ps", bufs=2, space="PSUM") as tr2_ps:

            for h in range(2):
                c0 = h * H
                att_acc = [
                    att_ps.tile([DK + 1, 512], F32, tag=f"att_acc{c}",
                                name=f"att_acc{h}_{c}")
                    for c in range(HCH)
                ]
                g_tiles = []
                for kb in range(NKB):
                    mtile = mask_p.tile([P, H], U8, tag="mask")
                    nc.sync.dma_start(
                        mtile[:, :], maskT[kb * P:(kb + 1) * P, c0:c0 + H]
                    )
                    ps = sc_ps.tile([P, H], F32, tag="sc_ps")
                    for cc in range(HCH):
                        nc.tensor.matmul(
                            ps[:, cc * 512:(cc + 1) * 512],
                            kT_sb[:, kb * P:(kb + 1) * P],
                            qT_sb[:, c0 + cc * 512:c0 + (cc + 1) * 512],
                        )
                    er = er_p.tile([P, H], BF16, tag="er")
                    nc.scalar.activation(er[:, :], ps[:, :], AF.Exp,
                                         bias=0.0, scale=SCALE)
                    g = et_pool.tile([P, H], BF16, tag="et", name=f"g_{h}_{kb}")
                    # g = (e - 1) * mask;  e == g + 1 everywhere we need it
                    nc.vector.scalar_tensor_tensor(
                        g[:, :], er[:, :], 1.0, mtile[:, :],
                        op0=OP.subtract, op1=OP.mult,
                    )
                    g_tiles.append(g)
                    for cc in range(HCH):
                        nc.tensor.matmul(
                            att_acc[cc][:, :],
                            v1_tiles[kb][:, :],
                            g[:, cc * 512:(cc + 1) * 512],
                            start=(kb == 0),
                            stop=(kb == NKB - 1),
                        )

                # ---- per-half epilogue ----
                att_sb = ph2.tile([DK + 1, H], F32, tag="att_sb")
                for cc in range(HCH):
                    nc.scalar.copy(att_sb[:, cc * 512:(cc + 1) * 512],
                                   att_acc[cc][:, :])
                # undo g = e - 1:  att += C (per-partition scalar)
                nc.vector.tensor_scalar_add(att_sb[:, :], att_sb[:, :],
                                            c_sb[:, :])

                # rowsums -> 1/r, broadcast across partitions via DRAM bounce
                nc.sync.dma_start(r_scr[c0:c0 + H], att_sb[DK:DK + 1, :])
                rr_t = ph2.tile([P, H // P], F32, tag="rr_t")
                nc.sync.dma_start(
                    rr_t[:, :], r_scr[c0:c0 + H].rearrange("(p f) -> p f", p=P)
                )
                rr_inv = ph2.tile([P, H // P], BF16, tag="rr_inv")
                with nc.allow_low_precision(reason="bf16 1/rowsum within tol"):
                    nc.vector.reciprocal(rr_inv[:, :], rr_t[:, :])
                nc.sync.dma_start(
                    rinv_scr[c0:c0 + H].rearrange("(p f) -> p f", p=P),
                    rr_inv[:, :],
                )
                rinv_bc = ph2.tile([P, H], BF16, tag="rinv_bc")
                nc.sync.dma_start(
                    rinv_bc[:, :],
                    rinv_scr[c0:c0 + H]
                    .rearrange("(a s) -> a s", a=1)
                    .to_broadcast((P, H)),
                )

                # attention weights out: w^T = (g + 1) * rinv, per sk block
                for kb in range(NKB):
                    wsb = w_p.tile([P, H], BF16, tag="wsb")
                    nc.vector.scalar_tensor_tensor(
                        wsb[:, :], g_tiles[kb][:, :], 1.0, rinv_bc[:, :],
                        op0=OP.add, op1=OP.mult,
                    )
                    nc.sync.dma_start(
                        w_t[kb * P:(kb + 1) * P, c0:c0 + H], wsb[:, :]
                    )

                # attention out: normalize attT, transpose back to [sq, 64]
                attn_sb = ph2.tile([DK, H], F32, tag="attn_sb")
                nc.vector.tensor_tensor(
                    attn_sb[:, :], att_sb[0:DK, :], rinv_bc[0:DK, :],
                    op=OP.mult,
                )
                for i in range(H // P):
                    pst = tr2_ps.tile([P, DK], F32, tag="tr2_ps")
                    nc.tensor.transpose(
                        pst[:, :], attn_sb[:, i * P:(i + 1) * P],
                        ident_f32[:DK, :DK],
                    )
                    ao = ao_p.tile([P, DK], F32, tag="ao")
                    nc.scalar.copy(ao[:, :], pst[:, :])
                    nc.sync.dma_start(
                        att[c0 + i * P:c0 + (i + 1) * P, :], ao[:, :]
                    )

    nc.finalize()
    return nc


_CACHE = {}


def _get_graph():
    if "nc" not in _CACHE:
        _CACHE["nc"] = build_graph()
    return _CACHE["nc"]


def make_in_maps(query, key, value, attention_mask, Wq, bq, Wk, bk, Wv, bv):
    query = np.asarray(query)
    key = np.asarray(key)
    value = np.asarray(value)
    attention_mask = np.asarray(attention_mask)
    shared = {
        "wqT": np.ascontiguousarray(np.asarray(Wq, np.float32).T).astype(NPBF16),
        "wkT": np.ascontiguousarray(np.asarray(Wk, np.float32).T).astype(NPBF16),
        "wvT": np.ascontiguousarray(np.asarray(Wv, np.float32).T).astype(NPBF16),
        "bq": np.asarray(bq, np.float32).reshape(DK, 1),
        "bk": np.asarray(bk, np.float32).reshape(DK, 1),
        "bv": np.asarray(bv, np.float32).reshape(DK, 1),
    }
    in_maps = []
    for b in range(B):
        in_maps.append(
            {
                "qT": np.ascontiguousarray(query[b].T).astype(NPBF16),
                "kT": np.ascontiguousarray(key[b].T).astype(NPBF16),
                "vT": np.ascontiguousarray(value[b].T).astype(NPBF16),
                "maskT": np.ascontiguousarray(attention_mask[b].T).astype(np.uint8),
                **shared,
            }
        )
    return in_maps


def kernel(query, key, value, attention_mask, Wq, bq, Wk, bk, Wv, bv):
    global LAST_RESULTS
    nc = _get_graph()
    in_maps = make_in_maps(
        query, key, value, attention_mask, Wq, bq, Wk, bk, Wv, bv
    )
    res = run_bass_kernel_spmd(nc, in_maps, core_ids=list(range(NCORES)))
    LAST_RESULTS = res
    att = np.stack(
        [np.asarray(res.results[c]["att"], np.float32) for c in range(NCORES)]
    )
    w = np.stack(
        [
            np.asarray(res.results[c]["w_t"]).astype(np.float32).T
            for c in range(NCORES)
        ]
    )
    return att, np.ascontiguousarray(w)


# revision 14
# speedup vs baseline: 1.2869x; 1.0929x over previous
"""Trainium2 Bass kernel for nn_Attention_50654844289068.

Strategy (8 NeuronCores, data-parallel over batch B=8 -> 1 batch element per core):

  reference math per batch b:
    q = query @ Wq.T + bq            (S, 64)
    k = key   @ Wk.T + bk            (S, 64)
    v = value @ Wv.T + bv            (S, 64)
    s = (q @ k.T) * scale            (S, S)
    s = where(s == 0, eps, s); s = where(mask == 0, eps, s)
    w = softmax(s, axis=-1)          (S, S)   <- output 2
    att = w @ v                      (S, 64)  <- output 1

  Device-side layout choices (per core):
    - All big tensors are handled in TRANSPOSED score layout  sT[sk, sq]
      so that softmax's reduction axis (sk) lands on the partition axis,
      where the TensorEngine can reduce it for free via an appended
      ones-column in the attention@V matmul, and the e^T tiles are directly
      usable as the stationary operand of that matmul (no on-chip 2048x2048
      transpose needed).
    - The host pre-transposes query/key/value ( -> [512, S]) and the mask
      ( -> maskT[sk, sq]) while sharding, and post-transposes the weight
      output (device writes w^T).  Host-side work is only layout/dtype prep.
    - masked_fill(s==0, eps) + masked_fill(mask==0, eps):  eps = 1e-6, and
      exp(1e-6) == 1 + 1e-6.  We instead compute e = exp(scale*s*mask) so
      masked lanes give exp(0) = 1 — a 1e-6 relative difference, far below
      tolerance.  softmax has no max-subtraction: scores*scale are O(+-2),
      exp is perfectly stable there (matches jax softmax mathematically).
    - bf16 compute on PE (fp32 matmul is 4x slower), fp32 PSUM accumulate.

  Per-core phases:
    P0: project q^T,k^T [64,S] (bf16) and v [S,64] (+ ones col) from
        host-transposed inputs.
    P1: for each of 16 sk-blocks: scoresT = k_blk^T . q  (PE) ->
        s' = (scores*scale)*mask (DVE, reads PSUM) -> eT = exp(s') (ACT)
        -> accumulate attT[65, S] += [v|1]^T . eT (PE).
    P2: rowsums = attT row 64 -> 1/r (DVE) -> broadcast over partitions via
        a tiny DRAM bounce -> w^T = eT * rinv (DVE) -> DMA out;
        att = (attT rows 0:64 * rinv) transposed back on PE -> DMA out.
"""

import os
import sys
from contextlib import ExitStack

sys.path.insert(0, "/opt/trn_rl_repo")

import numpy as np
import ml_dtypes

import concourse.bacc as bacc
import concourse.bass as bass
import concourse.tile as tile
from concourse import masks, mybir
from concourse.bass_utils import run_bass_kernel_spmd

B, S, DM, DK = 8, 2048, 512, 64
NCORES = 8
P = 128
NKB = S // P          # 16 sk blocks
NCH = S // 512        # 4 sq chunks of 512
SCALE = float(DK) ** -0.5

F32 = mybir.dt.float32
BF16 = mybir.dt.bfloat16
U8 = mybir.dt.uint8
NPBF16 = ml_dtypes.bfloat16

AF = mybir.ActivationFunctionType
OP = mybir.AluOpType

LAST_RESULTS = None


def build_graph():
    nc = bacc.Bacc(
        "TRN2",
        target_bir_lowering=False,
        debug=False,
        num_devices=NCORES,
    )

    qT = nc.declare_dram_parameter("qT", [DM, S], BF16, isOutput=False)
    kT = nc.declare_dram_parameter("kT", [DM, S], BF16, isOutput=False)
    vT = nc.declare_dram_parameter("vT", [DM, S], BF16, isOutput=False)
    maskT = nc.declare_dram_parameter("maskT", [S, S], U8, isOutput=False)
    wTs = {
        t: nc.declare_dram_parameter(f"w{t}T", [DM, DK], BF16, isOutput=False)
        for t in "qkv"
    }
    biases = {
        t: nc.declare_dram_parameter(f"b{t}", [P, 1], F32, isOutput=False)
        for t in "qkv"
    }
    w_t = nc.declare_dram_parameter("w_t", [S, S], BF16, isOutput=True)
    att_t = nc.declare_dram_parameter("att_t", [DK, S], F32, isOutput=True)

    r_scr = nc.dram_tensor("r_scr", [S], F32)
    rinv_scr = nc.dram_tensor("rinv_scr", [S], BF16)

    xTs = {"q": qT, "k": kT, "v": vT}

    with tile.TileContext(nc) as tc, ExitStack() as ctx:
        persist = ctx.enter_context(tc.tile_pool(name="persist", bufs=1))
        et_pool = ctx.enter_context(tc.tile_pool(name="et", bufs=2 * NKB))

        ident_bf16 = persist.tile([P, P], BF16, tag="ident_bf16")
        masks.make_identity(nc, ident_bf16[:, :])

        # q^T and k^T duplicated onto both partition halves [0:64) and
        # [64:128) so consecutive sk blocks can run as concurrent row-group
        # matmul tiles on the PE array (K=64 uses only half the rows).
        qT_sb = persist.tile([P, S], BF16, tag="qT_sb")
        kT_sb = persist.tile([P, S], BF16, tag="kT_sb")
        v1_tiles = [
            persist.tile([P, DK + 1], BF16, tag=f"v1_{i}", name=f"v1_{i}")
            for i in range(NKB)
        ]

        # ---------------- P0: projections ----------------
        with tc.tile_pool(name="pro_in", bufs=4) as pro_in, \
             tc.tile_pool(name="pro_w", bufs=1) as pro_w, \
             tc.tile_pool(name="pro_tmp", bufs=1) as pro_tmp, \
             tc.tile_pool(name="pro_ps", bufs=4, space="PSUM") as pro_ps, \
             tc.tile_pool(name="tr_ps", bufs=2, space="PSUM") as tr_ps:

            bias_sb = {}
            for t in "qkv":
                bt = pro_w.tile([P, 1], F32, tag=f"bias_{t}")
                nc.sync.dma_start(bt[:, :], biases[t][:, :])
                bias_sb[t] = bt

            vT_sb = pro_tmp.tile([DK, S], BF16, tag="vT_sb")
            dst = {"q": qT_sb, "k": kT_sb, "v": vT_sb}

            for t in "qkv":
                dup = t in "qk"   # duplicate onto partitions [64:128)
                w_tiles = []
                for m in range(4):
                    wt = pro_w.tile([P, DK], BF16, tag=f"w_{t}_{m}")
                    nc.sync.dma_start(wt[:, :], wTs[t][m * P:(m + 1) * P, :])
                    w_tiles.append(wt)
                pss = [pro_ps.tile([P, 512], F32, tag="proj_ps", name=f"ps_{t}_{c}")
                       for c in range(NCH)]
                for m in range(4):
                    xt = pro_in.tile([P, S], BF16, tag="xin")
                    nc.sync.dma_start(xt[:, :], xTs[t][m * P:(m + 1) * P, :])
                    for c in range(NCH):
                        nc.tensor.matmul(
                            pss[c][0:DK, :],
                            w_tiles[m][:, :],
                            xt[:, c * 512:(c + 1) * 512],
                            start=(m == 0),
                            stop=(m == 3),
                            tile_position=(0, 0),
                            skip_group_check=True,
                        )
                        if dup:
                            nc.tensor.matmul(
                                pss[c][DK:2 * DK, :],
                                w_tiles[m][:, :],
                                xt[:, c * 512:(c + 1) * 512],
                                start=(m == 0),
                                stop=(m == 3),
                                tile_position=(0, DK),
                                skip_group_check=True,
                            )
                for c in range(NCH):
                    if dup:
                        nc.scalar.activation(
                            dst[t][:, c * 512:(c + 1) * 512],
                            pss[c][:, :],
                            AF.Identity,
                            bias=bias_sb[t][:, :],
                            scale=1.0,
                        )
                    else:
                        nc.scalar.activation(
                            dst[t][:, c * 512:(c + 1) * 512],
                            pss[c][0:DK, :],
                            AF.Identity,
                            bias=bias_sb[t][0:DK, :],
                            scale=1.0,
                        )

            # v natural [sk, 64] tiles with an appended ones column
            for i in range(NKB):
                pst = tr_ps.tile([P, DK], BF16, tag="tr_ps")
                nc.tensor.transpose(
                    pst[:, :], vT_sb[:, i * P:(i + 1) * P], ident_bf16[:DK, :DK]
                )
                nc.scalar.copy(v1_tiles[i][:, 0:DK], pst[:, :])
                nc.vector.memset(v1_tiles[i][:, DK:DK + 1], 1.0)


        # ---------------- main: two sq halves, pipelined ----------------
        H = S // 2          # 1024 columns (sq) per half
        HCH = H // 512      # 2 chunks of 512
        with tc.tile_pool(name="att_ps", bufs=1, space="PSUM") as att_ps, \
             tc.tile_pool(name="mask_p", bufs=4) as mask_p, \
             tc.tile_pool(name="sp_p", bufs=3) as sp_p, \
             tc.tile_pool(name="sc_ps", bufs=3, space="PSUM") as sc_ps, \
             tc.tile_pool(name="ph2", bufs=2) as ph2, \
             tc.tile_pool(name="w_p", bufs=4) as w_p:

            for h in range(2):
                c0 = h * H
                att_acc = [
                    att_ps.tile([DK + 1, 512], F32, tag=f"att_acc{c}",
                                name=f"att_acc{h}_{c}")
                    for c in range(HCH)
                ]
                e_tiles = []
                for kb in range(NKB):
                    mtile = mask_p.tile([P, H], U8, tag="mask")
                    nc.sync.dma_start(
                        mtile[:, :], maskT[kb * P:(kb + 1) * P, c0:c0 + H]
                    )
                    # consecutive kb alternate PE row groups (K=64): the two
                    # matmul streams run concurrently on the half-filled array
                    rg = DK * (kb % 2)
                    ps = sc_ps.tile([P, H], F32, tag="sc_ps")
                    for cc in range(HCH):
                        nc.tensor.matmul(
                            ps[:, cc * 512:(cc + 1) * 512],
                            kT_sb[rg:rg + DK, kb * P:(kb + 1) * P],
                            qT_sb[rg:rg + DK, c0 + cc * 512:c0 + (cc + 1) * 512],
                            tile_position=(rg, 0),
                        )
                    # s' = (scores * scale) * mask   (PSUM + SBUF -> SBUF bf16)
                    sp = sp_p.tile([P, H], BF16, tag="sp")
                    nc.vector.scalar_tensor_tensor(
                        sp[:, :], ps[:, :], SCALE, mtile[:, :],
                        op0=OP.mult, op1=OP.mult,
                    )
                    e = et_pool.tile([P, H], BF16, tag="et", name=f"e_{h}_{kb}")
                    nc.scalar.activation(e[:, :], sp[:, :], AF.Exp,
                                         bias=0.0, scale=1.0)
                    e_tiles.append(e)
                    for cc in range(HCH):
                        nc.tensor.matmul(
                            att_acc[cc][:, :],
                            v1_tiles[kb][:, :],
                            e[:, cc * 512:(cc + 1) * 512],
                            start=(kb == 0),
                            stop=(kb == NKB - 1),
                        )

                # ---- per-half epilogue ----
                att_sb = ph2.tile([DK + 1, H], F32, tag="att_sb")
                for cc in range(HCH):
                    nc.scalar.copy(att_sb[:, cc * 512:(cc + 1) * 512],
                                   att_acc[cc][:, :])

                # rowsums -> 1/r, broadcast across partitions via DRAM bounce
                nc.sync.dma_start(r_scr[c0:c0 + H], att_sb[DK:DK + 1, :])
                rr_t = ph2.tile([P, H // P], F32, tag="rr_t")
                nc.sync.dma_start(
                    rr_t[:, :], r_scr[c0:c0 + H].rearrange("(p f) -> p f", p=P)
                )
                rr_inv = ph2.tile([P, H // P], BF16, tag="rr_inv")
                with nc.allow_low_precision(reason="bf16 1/rowsum within tol"):
                    nc.vector.reciprocal(rr_inv[:, :], rr_t[:, :])
                nc.sync.dma_start(
                    rinv_scr[c0:c0 + H].rearrange("(p f) -> p f", p=P),
                    rr_inv[:, :],
                )
                rinv_bc = ph2.tile([P, H], BF16, tag="rinv_bc")
                nc.sync.dma_start(
                    rinv_bc[:, :],
                    rinv_scr[c0:c0 + H]
                    .rearrange("(a s) -> a s", a=1)
                    .to_broadcast((P, H)),
                )

                # attention weights out: w^T = e * rinv, per sk block.
                # Alternate DVE / GpSimd so both elementwise engines pull.
                for kb in range(NKB):
                    wsb = w_p.tile([P, H], BF16, tag="wsb")
                    eng = nc.gpsimd if kb % 2 == 1 else nc.vector
                    eng.tensor_tensor(
                        wsb[:, :], e_tiles[kb][:, :], rinv_bc[:, :], op=OP.mult
                    )
                    nc.sync.dma_start(
                        w_t[kb * P:(kb + 1) * P, c0:c0 + H], wsb[:, :]
                    )

                # attention out, transposed ([64, sq]); host transposes back
                attn_sb = ph2.tile([DK, H], F32, tag="attn_sb")
                nc.vector.tensor_tensor(
                    attn_sb[:, :], att_sb[0:DK, :], rinv_bc[0:DK, :],
                    op=OP.mult,
                )
                nc.sync.dma_start(att_t[:, c0:c0 + H], attn_sb[:, :])

    nc.finalize()
    return nc


_CACHE = {}


def _get_graph():
    if "nc" not in _CACHE:
        _CACHE["nc"] = build_graph()
    return _CACHE["nc"]


def make_in_maps(query, key, value, attention_mask, Wq, bq, Wk, bk, Wv, bv):
    query = np.asarray(query)
    key = np.asarray(key)
    value = np.asarray(value)
    attention_mask = np.asarray(attention_mask)
    shared = {
        "wqT": np.ascontiguousarray(np.asarray(Wq, np.float32).T).astype(NPBF16),
        "wkT": np.ascontiguousarray(np.asarray(Wk, np.float32).T).astype(NPBF16),
        "wvT": np.ascontiguousarray(np.asarray(Wv, np.float32).T).astype(NPBF16),
        "bq": np.tile(np.asarray(bq, np.float32), 2).reshape(P, 1),
        "bk": np.tile(np.asarray(bk, np.float32), 2).reshape(P, 1),
        "bv": np.tile(np.asarray(bv, np.float32), 2).reshape(P, 1),
    }
    in_maps = []
    for b in range(B):
        in_maps.append(
            {
                "qT": np.ascontiguousarray(query[b].T).astype(NPBF16),
                "kT": np.ascontiguousarray(key[b].T).astype(NPBF16),
                "vT": np.ascontiguousarray(value[b].T).astype(NPBF16),
                "maskT": np.ascontiguousarray(attention_mask[b].T).astype(np.uint8),
                **shared,
            }
        )
    return in_maps


def kernel(query, key, value, attention_mask, Wq, bq, Wk, bk, Wv, bv):
    global LAST_RESULTS
    nc = _get_graph()
    in_maps = make_in_maps(
        query, key, value, attention_mask, Wq, bq, Wk, bk, Wv, bv
    )
    res = run_bass_kernel_spmd(nc, in_maps, core_ids=list(range(NCORES)))
    LAST_RESULTS = res
    att = np.stack(
        [
            np.asarray(res.results[c]["att_t"], np.float32).T
            for c in range(NCORES)
        ]
    )
    w = np.stack(
        [
            np.asarray(res.results[c]["w_t"]).astype(np.float32).T
            for c in range(NCORES)
        ]
    )
    return np.ascontiguousarray(att), np.ascontiguousarray(w)
